# revision 14
# baseline (speedup 1.0000x reference)
"""Trainium2 Bass kernel for nn_ConformerBlock — self-contained.

Strategy: data-parallel over batch (16 batches / 8 cores = 2 per core).
Inside each core the block runs with TRANSPOSED activations
([channels-on-partitions, positions-on-free]) so every matmul keeps the
weight stationary and no per-stage activation transposes are needed.

Host-side preprocessing (inside kernel()):
  - LN gamma folded into following weight matrices; LN beta folded into the
    following bias vector.
  - attention score scale folded into Wq.
  - BN (inference) folded into depthwise conv weights + bias.
  - rel_emb reversed/transposed so the relative-position gather becomes a
    strided (contiguous-run) DMA read from a DRAM skew buffer.

Relative attention: u'[i, r] = q[i] . E[1024 - r] is computed per q-tile as
a matmul window, written to DRAM with row pitch 1024, and read back with row
pitch 1023 — the off-by-one pitch realizes rel[i, j] = q[i] . E[i - j + 512]
(the classic skew trick), then added into the score PSUM with an
identity-weight matmul. Softmax uses exp without max subtraction (scores are
O(1) here) with the row sum taken by the activation engine's accumulator.
"""

import os
import sys

import numpy as np

if "/opt/trn_rl_repo" not in sys.path:
    sys.path.insert(0, "/opt/trn_rl_repo")

import ml_dtypes

BF = ml_dtypes.bfloat16

# problem dims
B, N, D = 16, 512, 256
H, DH = 4, 64
MAXP = 512
FF = 4 * D          # 1024
IC = 2 * D          # 512
KW = 31             # conv kernel width
EPS = 1e-5
SF = 1.0

NCORES = 8
BL = B // NCORES    # batches per core = 2
CT = D // 128       # 2 channel tiles of the model dim
NT = N // 128       # 4 position tiles
FT = FF // 128      # 8 ff tiles
GT = IC // 128      # 4 glu/conv channel tiles

_BUILD_CACHE = {}


def _host_prep(inputs):
    """Fold/cast/layout all weights. Returns dict of NEFF input arrays
    (shared across cores) plus flags that gate optional bias matmuls."""
    f = {k: np.asarray(v, np.float32) for k, v in inputs.items()}
    scale = DH ** -0.5

    def colmajor(vec, ntile):
        # [ntile*128] -> [128, ntile] (per-partition bias columns)
        return np.ascontiguousarray(vec.reshape(ntile, 128).T).astype(np.float32)

    p = {}
    flags = {}

    # FF1
    w1 = f["ff1_ln_g"][:, None] * f["ff1_w1"]
    b1 = f["ff1_ln_b"] @ f["ff1_w1"] + f["ff1_b1"]
    p["w1"] = w1.astype(BF)
    p["b1c"] = colmajor(b1, FT)
    p["w2"] = f["ff1_w2"].astype(BF)
    p["b2r"] = f["ff1_b2"][None, :].astype(BF)
    flags["b2"] = bool(np.any(f["ff1_b2"] != 0))

    # attention
    qkvw = f["attn_ln_g"][:, None] * f["qkv_w"]
    qkvb = f["attn_ln_b"] @ f["qkv_w"]
    qkvw[:, : H * DH] *= scale
    qkvb[: H * DH] *= scale
    p["qkvw"] = qkvw.astype(BF)
    p["qkvbr"] = qkvb[None, :].astype(BF)
    flags["qkvb"] = bool(np.any(qkvb != 0))
    p["outw"] = f["out_w"].astype(BF)
    p["outbr"] = f["out_b"][None, :].astype(BF)
    flags["outb"] = bool(np.any(f["out_b"] != 0))
    # ETrev[:, r] = E[1024 - r]; duplicated across both 64-partition halves
    erevT = np.ascontiguousarray(f["rel_emb"][::-1].T)        # [64, 1025]
    p["etr"] = np.concatenate([erevT, erevT], axis=0).astype(BF)  # [128, 1025]

    # conv module
    pw1w = f["conv_ln_g"][:, None] * f["pw1_w"]
    pw1b = f["conv_ln_b"] @ f["pw1_w"] + f["pw1_b"]
    p["pw1w"] = pw1w.astype(BF)
    p["pw1bc"] = colmajor(pw1b, 2 * GT)                       # [128, 8]
    bn_scale = f["bn_g"] / np.sqrt(f["bn_var"] + EPS)
    dw = f["dw_w"][:, 0, :] * bn_scale[:, None]               # [IC, KW]
    db = (f["dw_b"] - f["bn_mean"]) * bn_scale + f["bn_b"]
    p["dwc"] = np.ascontiguousarray(
        dw.reshape(GT, 128, KW).transpose(1, 0, 2)).astype(np.float32)  # [128,GT,KW]
    p["dbc"] = colmajor(db, GT)                               # [128, 4]
    p["pw2w"] = f["pw2_w"].astype(BF)
    p["pw2br"] = f["pw2_b"][None, :].astype(BF)
    flags["pw2b"] = bool(np.any(f["pw2_b"] != 0))

    # FF2
    fw1 = f["ff2_ln_g"][:, None] * f["ff2_w1"]
    fb1 = f["ff2_ln_b"] @ f["ff2_w1"] + f["ff2_b1"]
    p["f2w1"] = fw1.astype(BF)
    p["f2b1c"] = colmajor(fb1, FT)
    p["f2w2"] = f["ff2_w2"].astype(BF)
    p["f2b2r"] = f["ff2_b2"][None, :].astype(BF)
    flags["f2b2"] = bool(np.any(f["ff2_b2"] != 0))

    # post norm affine
    p["pngc"] = colmajor(f["pn_g"], CT)
    p["pnbc"] = colmajor(f["pn_b"], CT)
    return p, flags


def _build(flags, conv_pe_taps, conv_gp_taps, sim_compat=False):
    """Emit the Bass/Tile program. Returns nc."""
    import concourse.bass as bass
    import concourse.bacc as bacc
    import concourse.mybir as mybir
    import concourse.tile as tile
    from concourse.masks import make_identity

    dt = mybir.dt
    AF = mybir.ActivationFunctionType
    OP = mybir.AluOpType

    nc = bacc.Bacc("TRN2", target_bir_lowering=False, debug=False)

    # ---------------- DRAM tensors ----------------
    xin = nc.dram_tensor("xin", [BL, N, D], dt.float32, kind="ExternalInput")
    out = nc.dram_tensor("out", [BL, N, D], dt.float32, kind="ExternalOutput")

    def din(name, shape, dtype=dt.bfloat16):
        return nc.dram_tensor(name, shape, dtype, kind="ExternalInput")

    w1_d = din("w1", [D, FF]); b1c_d = din("b1c", [128, FT], dt.float32)
    w2_d = din("w2", [FF, D]); b2r_d = din("b2r", [1, D])
    qkvw_d = din("qkvw", [D, 3 * H * DH]); qkvbr_d = din("qkvbr", [1, 3 * H * DH])
    outw_d = din("outw", [D, D]); outbr_d = din("outbr", [1, D])
    etr_d = din("etr", [128, 2 * MAXP + 1])
    pw1w_d = din("pw1w", [D, 2 * IC]); pw1bc_d = din("pw1bc", [128, 2 * GT], dt.float32)
    dwc_d = din("dwc", [128, GT, KW], dt.float32)
    dbc_d = din("dbc", [128, GT], dt.float32)
    pw2w_d = din("pw2w", [IC, D]); pw2br_d = din("pw2br", [1, D])
    f2w1_d = din("f2w1", [D, FF]); f2b1c_d = din("f2b1c", [128, FT], dt.float32)
    f2w2_d = din("f2w2", [FF, D]); f2b2r_d = din("f2b2r", [1, D])
    pngc_d = din("pngc", [128, CT], dt.float32)
    pnbc_d = din("pnbc", [128, CT], dt.float32)

    # skew scratch: one slot per (b, h, itile): [128 rows, pitch 1024] bf16
    skew = nc.dram_tensor("skew", [BL * H * NT, 128, 1024], dt.bfloat16)

    f32r = dt.float32r

    with tile.TileContext(nc) as tc:
        with (
            tc.tile_pool(name="consts", bufs=1) as consts,
            tc.tile_pool(name="res", bufs=6) as res_pool,
            tc.tile_pool(name="tmp", bufs=4) as tmp_pool,
            tc.tile_pool(name="act", bufs=6) as act_pool,
            tc.tile_pool(name="mid", bufs=10) as mid_pool,
            tc.tile_pool(name="qkv", bufs=6) as qkv_pool,
            tc.tile_pool(name="vp", bufs=6) as v_pool,
            tc.tile_pool(name="pp", bufs=10) as p_pool,
            tc.tile_pool(name="up", bufs=3) as u_pool,
            tc.tile_pool(name="cv", bufs=6) as cv_pool,
            tc.tile_pool(name="st", bufs=8) as st_pool,
            tc.tile_pool(name="xio", bufs=4) as xio_pool,
            tc.tile_pool(name="ps", bufs=4, space="PSUM") as ps_pool,
            tc.tile_pool(name="psu", bufs=2, space="PSUM") as psu_pool,
        ):
            # ---------------- constants / weights to SBUF ----------------
            def wload(dram, shape, rearr=None):
                t = consts.tile(shape, dram.dtype, name=dram.name + "_s",
                                tag=dram.name + "_s")
                src = (dram.ap() if rearr is None
                       else dram.ap().rearrange(rearr, p=128))
                nc.sync.dma_start(out=t, in_=src)
                return t

            w1_s = wload(w1_d, [128, CT, FF], "(kt p) m -> p kt m")
            w2_s = wload(w2_d, [128, FT, D], "(kt p) m -> p kt m")
            qkvw_s = wload(qkvw_d, [128, CT, 3 * H * DH], "(kt p) m -> p kt m")
            outw_s = wload(outw_d, [128, CT, D], "(kt p) m -> p kt m")
            etr_s = wload(etr_d, [128, 2 * MAXP + 1])
            pw1w_s = wload(pw1w_d, [128, CT, 2 * IC], "(kt p) m -> p kt m")
            pw2w_s = wload(pw2w_d, [128, GT, D], "(kt p) m -> p kt m")
            f2w1_s = wload(f2w1_d, [128, CT, FF], "(kt p) m -> p kt m")
            f2w2_s = wload(f2w2_d, [128, FT, D], "(kt p) m -> p kt m")
            b1c_s = wload(b1c_d, [128, FT])
            f2b1c_s = wload(f2b1c_d, [128, FT])
            pw1bc_s = wload(pw1bc_d, [128, 2 * GT])
            dwc_s = wload(dwc_d, [128, GT, KW])
            dbc_s = wload(dbc_d, [128, GT])
            pngc_s = wload(pngc_d, [128, CT])
            pnbc_s = wload(pnbc_d, [128, CT])
            b2r_s = wload(b2r_d, [1, D]) if flags["b2"] else None
            qkvbr_s = wload(qkvbr_d, [1, 3 * H * DH]) if flags["qkvb"] else None
            outbr_s = wload(outbr_d, [1, D]) if flags["outb"] else None
            pw2br_s = wload(pw2br_d, [1, D]) if flags["pw2b"] else None
            f2b2r_s = wload(f2b2r_d, [1, D]) if flags["f2b2"] else None

            ident_f = consts.tile([128, 128], dt.float32)
            make_identity(nc, ident_f)
            ident_b = consts.tile([128, 128], dt.bfloat16)
            make_identity(nc, ident_b)
            ones_col = consts.tile([128, 1], dt.float32)   # LN sum weights
            nc.vector.memset(ones_col, 1.0 / D)
            ones_colb = consts.tile([128, 1], dt.bfloat16)
            nc.vector.memset(ones_colb, 1.0 / D)
            ones_r1 = consts.tile([1, 128], dt.float32)    # broadcast lhsT
            nc.vector.memset(ones_r1, 1.0)
            ones_rb = consts.tile([1, N], dt.bfloat16)     # bias-matmul rhs
            nc.vector.memset(ones_rb, 1.0)
            eps_t = consts.tile([1, 1], dt.float32)
            nc.vector.memset(eps_t, EPS)
            ones_cb = consts.tile([1, 128], dt.bfloat16)
            nc.vector.memset(ones_cb, 1.0)

            def mm(ps, lhsT, rhs, start, stop, tp=None):
                nc.tensor.matmul(ps, lhsT, rhs, start=start, stop=stop,
                                 tile_position=tp)

            def emit_swish(out_t, src_ap, bias_ap):
                # out = silu(src + bias); sim_compat decomposes via sigmoid
                if not sim_compat:
                    nc.scalar.activation(out_t, src_ap, AF.Silu, bias=bias_ap)
                else:
                    sgt = tmp_pool.tile(list(out_t.shape), dt.bfloat16,
                                        tag="swt", name="swt")
                    nc.scalar.activation(sgt, src_ap, AF.Sigmoid, bias=bias_ap)
                    nc.vector.scalar_tensor_tensor(
                        out=out_t, in0=src_ap, scalar=bias_ap, in1=sgt,
                        op0=OP.add, op1=OP.mult)

            # ---------------- layer norm ----------------
            def layer_norm(x_ct, out_dtype, affine=None, precise=False):
                """x_ct: list of CT fp32 [128, N] tiles (transposed layout).
                Returns CT tiles [128, N] of out_dtype holding
                (x - mean) * rstd (optionally * g + b per-partition)."""
                s1 = ps_pool.tile([1, N], dt.float32, tag="ps")
                s2 = ps_pool.tile([1, N], dt.float32, tag="ps")
                sqs = []
                for ct in range(CT):
                    sq = tmp_pool.tile([128, N], dt.bfloat16, tag="tmp")
                    nc.vector.tensor_mul(sq, x_ct[ct], x_ct[ct])
                    sqs.append(sq)
                for ct in range(CT):
                    # fp32 matmul (4 cyc/row) — mean needs full precision
                    mm(s1, ones_col, x_ct[ct], ct == 0, ct == CT - 1)
                for ct in range(CT):
                    mm(s2, ones_colb, sqs[ct], ct == 0, ct == CT - 1)
                s1_sb = st_pool.tile([1, N], dt.float32, tag="st")
                nc.vector.tensor_copy(out=s1_sb, in_=s1)
                m2 = st_pool.tile([1, N], dt.float32, tag="st")
                nc.vector.tensor_mul(m2, s1_sb, s1_sb)
                var = st_pool.tile([1, N], dt.float32, tag="st")
                nc.vector.tensor_sub(var, s2, m2)
                std = st_pool.tile([1, N], dt.float32, tag="st")
                nc.scalar.activation(std, var, AF.Sqrt, bias=eps_t[:])
                rinv = st_pool.tile([1, N], dt.float32, tag="st")
                nc.vector.reciprocal(rinv, std)
                bdt = dt.float32 if precise else dt.bfloat16
                mean_s = st_pool.tile([1, N], bdt, tag="stm")
                nc.vector.tensor_copy(out=mean_s, in_=s1_sb)
                rinv_c = st_pool.tile([1, N], bdt, tag="stm")
                nc.vector.tensor_copy(out=rinv_c, in_=rinv)
                bones = ones_r1 if precise else ones_cb
                m_bc = ps_pool.tile([128, N], dt.float32, tag="ps")
                mm(m_bc, bones, mean_s, True, True)
                r_bc = ps_pool.tile([128, N], dt.float32, tag="ps")
                mm(r_bc, bones, rinv_c, True, True)
                outs = []
                for ct in range(CT):
                    t = tmp_pool.tile([128, N], dt.float32, tag="tmp")
                    nc.vector.tensor_sub(t, x_ct[ct], m_bc)
                    if affine is None:
                        h = act_pool.tile([128, N], out_dtype, tag="h")
                        nc.vector.tensor_mul(h, t, r_bc)
                    else:
                        t2 = tmp_pool.tile([128, N], dt.float32, tag="tmp")
                        nc.vector.tensor_mul(t2, t, r_bc)
                        g_ap, b_ap = affine
                        h = act_pool.tile([128, N], out_dtype, tag="h")
                        nc.vector.tensor_scalar(
                            out=h, in0=t2,
                            scalar1=g_ap[:, ct:ct + 1], scalar2=b_ap[:, ct:ct + 1],
                            op0=OP.mult, op1=OP.add)
                    outs.append(h)
                return outs

            # ---------------- feed forward (ff1/ff2) ----------------
            def feed_forward2(x_ct, w1s, b1cs, w2s, b2rs):
                h = layer_norm(x_ct, dt.bfloat16)
                mids = []
                for mt in range(FT):
                    ps = ps_pool.tile([128, N], dt.float32, tag="ps")
                    for kt in range(CT):
                        mm(ps, w1s[:, kt, mt * 128:(mt + 1) * 128], h[kt],
                           kt == 0, kt == CT - 1)
                    mid = mid_pool.tile([128, N], dt.bfloat16, tag="mid")
                    emit_swish(mid, ps, b1cs[:, mt:mt + 1])
                    mids.append(mid)
                x_new = []
                for ct in range(CT):
                    ps = ps_pool.tile([128, N], dt.float32, tag="ps")
                    nmm = FT + (1 if b2rs is not None else 0)
                    idx = 0
                    for mt in range(FT):
                        idx += 1
                        mm(ps, w2s[:, mt, ct * 128:(ct + 1) * 128], mids[mt],
                           mt == 0, idx == nmm)
                    if b2rs is not None:
                        mm(ps, b2rs[:, ct * 128:(ct + 1) * 128], ones_rb,
                           False, True)
                    xn = res_pool.tile([128, N], dt.float32, tag="res")
                    nc.vector.scalar_tensor_tensor(
                        out=xn, in0=ps, scalar=0.5 * SF, in1=x_ct[ct],
                        op0=OP.mult, op1=OP.add)
                    x_new.append(xn)
                return x_new

            # ---------------- per-batch program ----------------
            for b in range(BL):
                # ---- load + transpose input ----
                xT = [res_pool.tile([128, N], dt.float32, tag="res",
                                    name=f"xT_{b}_{i}") for i in range(CT)]
                for nt in range(NT):
                    xna = xio_pool.tile([128, D], dt.float32, tag="xio")
                    nc.sync.dma_start(
                        out=xna, in_=xin.ap()[b, nt * 128:(nt + 1) * 128, :])
                    for ct in range(CT):
                        pst = ps_pool.tile([128, 128], dt.float32, tag="ps")
                        nc.tensor.transpose(
                            pst, xna[:, ct * 128:(ct + 1) * 128], ident_f)
                        nc.vector.tensor_copy(
                            out=xT[ct][:, nt * 128:(nt + 1) * 128], in_=pst)

                # ---- FF1 ----
                x1 = feed_forward2(xT, w1_s, b1c_s, w2_s, b2r_s)

                # ---- attention ----
                h2 = layer_norm(x1, dt.bfloat16)
                # q/k tiles: [128 (2 heads x 64), N]
                qk_sb = []
                for mt in range(4):
                    ps = ps_pool.tile([128, N], dt.float32, tag="ps")
                    nmm = CT + (1 if qkvbr_s is not None else 0)
                    idx = 0
                    for kt in range(CT):
                        idx += 1
                        mm(ps, qkvw_s[:, kt, mt * 128:(mt + 1) * 128], h2[kt],
                           kt == 0, idx == nmm)
                    if qkvbr_s is not None:
                        mm(ps, qkvbr_s[:, mt * 128:(mt + 1) * 128], ones_rb,
                           False, True)
                    t = qkv_pool.tile([128, N], dt.bfloat16, tag="qk")
                    nc.vector.tensor_copy(out=t, in_=ps)
                    qk_sb.append(t)
                # v tiles: [128 j, 256 d'] natural layout
                v_sb = []
                for jt in range(NT):
                    ps = ps_pool.tile([128, H * DH], dt.float32, tag="ps")
                    nmm = CT + (1 if qkvbr_s is not None else 0)
                    idx = 0
                    for kt in range(CT):
                        idx += 1
                        mm(ps, h2[kt][:, jt * 128:(jt + 1) * 128],
                           qkvw_s[:, kt, 2 * H * DH:3 * H * DH],
                           kt == 0, idx == nmm)
                    if qkvbr_s is not None:
                        mm(ps, ones_cb,
                           qkvbr_s[:, 2 * H * DH:3 * H * DH],
                           False, True)
                    t = v_pool.tile([128, H * DH], dt.bfloat16, tag="v")
                    nc.vector.tensor_copy(out=t, in_=ps)
                    v_sb.append(t)

                for hp in range(2):           # head pairs
                    ps_o = ps_pool.tile([128, N], dt.float32, tag="ps")
                    for hh in range(2):       # head within pair
                        h_id = hp * 2 + hh
                        pb = 64 * hh          # partition base
                        qt = qk_sb[hp]
                        kt_ = qk_sb[2 + hp]
                        z_t = st_pool.tile([128, NT], dt.float32, tag="zt")
                        p_n = []
                        for it in range(NT):
                            ps_s = ps_pool.tile([128, N], dt.float32, tag="ps")
                            mm(ps_s,
                               qt[pb:pb + 64, it * 128:(it + 1) * 128],
                               kt_[pb:pb + 64, :], True, False, tp=(pb, 0))
                            # qrel window
                            r0 = 385 - 128 * it
                            ps_u = psu_pool.tile([128, 640], dt.float32,
                                                 tag="psu")
                            mm(ps_u[:, 0:512],
                               qt[pb:pb + 64, it * 128:(it + 1) * 128],
                               etr_s[pb:pb + 64, r0:r0 + 512],
                               True, True, tp=(pb, 0))
                            mm(ps_u[:, 512:640],
                               qt[pb:pb + 64, it * 128:(it + 1) * 128],
                               etr_s[pb:pb + 64, r0 + 512:r0 + 640],
                               True, True, tp=(pb, 0))
                            u_sb = u_pool.tile([128, 640], dt.bfloat16,
                                               tag="u")
                            nc.vector.tensor_copy(out=u_sb, in_=ps_u)
                            slot = (b * H + h_id) * NT + it
                            nc.sync.dma_start(
                                out=skew.ap()[slot, :, 0:640], in_=u_sb)
                            rel_sb = u_pool.tile([128, N], dt.bfloat16,
                                                 tag="rel")
                            nc.sync.dma_start(
                                out=rel_sb,
                                in_=bass.AP(skew, slot * 131072 + 127,
                                            [[1023, 128], [1, N]]))
                            mm(ps_s, ident_b, rel_sb, False, True)
                            p_t = p_pool.tile([128, N], dt.bfloat16, tag="p")
                            nc.scalar.activation(
                                p_t, ps_s, AF.Exp,
                                accum_out=z_t[:, it:it + 1])
                            p_n.append(p_t)
                        rz = st_pool.tile([128, NT], dt.float32, tag="zt")
                        nc.vector.reciprocal(rz, z_t)
                        pT = [p_pool.tile([128, N], dt.bfloat16, tag="pT",
                                          name=f"pT_{b}_{h_id}_{i}")
                              for i in range(NT)]
                        for it in range(NT):
                            pn = p_pool.tile([128, N], dt.bfloat16, tag="pn")
                            nc.vector.tensor_scalar_mul(
                                pn, p_n[it], rz[:, it:it + 1])
                            for jt in range(NT):
                                nc.scalar.dma_start_transpose(
                                    out=pT[jt][:, it * 128:(it + 1) * 128],
                                    in_=pn[:, jt * 128:(jt + 1) * 128])
                        for jt in range(NT):
                            mm(ps_o[pb:pb + 64, :],
                               v_sb[jt][:, h_id * 64:(h_id + 1) * 64],
                               pT[jt], jt == 0, jt == NT - 1, tp=(0, pb))
                    o_sb = qkv_pool.tile([128, N], dt.bfloat16, tag="o")
                    nc.vector.tensor_copy(out=o_sb, in_=ps_o)
                    if hp == 0:
                        o_tiles = [o_sb]
                    else:
                        o_tiles.append(o_sb)

                x2 = []
                for et in range(CT):
                    ps = ps_pool.tile([128, N], dt.float32, tag="ps")
                    nmm = 2 + (1 if outbr_s is not None else 0)
                    idx = 0
                    for hp in range(2):
                        idx += 1
                        mm(ps, outw_s[:, hp, et * 128:(et + 1) * 128],
                           o_tiles[hp], hp == 0, idx == nmm)
                    if outbr_s is not None:
                        mm(ps, outbr_s[:, et * 128:(et + 1) * 128], ones_rb,
                           False, True)
                    xn = res_pool.tile([128, N], dt.float32, tag="res")
                    nc.vector.tensor_add(xn, ps, x1[et])
                    x2.append(xn)

                # ---- conv module ----
                h3 = layer_norm(x2, dt.bfloat16)
                sig = []
                ps_a = []
                for gt in range(GT):
                    psg = ps_pool.tile([128, N], dt.float32, tag="ps")
                    mt = GT + gt
                    for kt in range(CT):
                        mm(psg, pw1w_s[:, kt, mt * 128:(mt + 1) * 128],
                           h3[kt], kt == 0, kt == CT - 1)
                    sg = cv_pool.tile([128, N], dt.bfloat16, tag="sig")
                    nc.scalar.activation(sg, psg, AF.Sigmoid,
                                         bias=pw1bc_s[:, mt:mt + 1])
                    sig.append(sg)
                conv_in = []
                for gt in range(GT):
                    psa = ps_pool.tile([128, N], dt.float32, tag="ps")
                    for kt in range(CT):
                        mm(psa, pw1w_s[:, kt, gt * 128:(gt + 1) * 128],
                           h3[kt], kt == 0, kt == CT - 1)
                    cpad = cv_pool.tile([128, N + KW - 1], dt.bfloat16,
                                        tag="cpad")
                    nc.vector.memset(cpad[:, 0:KW // 2], 0.0)
                    nc.vector.memset(cpad[:, N + KW // 2:N + KW - 1], 0.0)
                    nc.vector.scalar_tensor_tensor(
                        out=cpad[:, KW // 2:KW // 2 + N], in0=psa,
                        scalar=pw1bc_s[:, gt:gt + 1], in1=sig[gt],
                        op0=OP.add, op1=OP.mult)
                    conv_in.append(cpad)
                conv_out = []
                for gt in range(GT):
                    acc = cv_pool.tile([128, N], dt.bfloat16, tag="acc")
                    nc.vector.tensor_scalar(
                        out=acc, in0=conv_in[gt][:, 0:N],
                        scalar1=dwc_s[:, gt, 0:1], scalar2=None, op0=OP.mult)
                    for kk in range(1, KW):
                        nc.vector.scalar_tensor_tensor(
                            out=acc, in0=conv_in[gt][:, kk:kk + N],
                            scalar=dwc_s[:, gt, kk:kk + 1], in1=acc,
                            op0=OP.mult, op1=OP.add)
                    co = cv_pool.tile([128, N], dt.bfloat16, tag="co")
                    emit_swish(co, acc, dbc_s[:, gt:gt + 1])
                    conv_out.append(co)
                x3 = []
                for et in range(CT):
                    ps = ps_pool.tile([128, N], dt.float32, tag="ps")
                    nmm = GT + (1 if pw2br_s is not None else 0)
                    idx = 0
                    for gt in range(GT):
                        idx += 1
                        mm(ps, pw2w_s[:, gt, et * 128:(et + 1) * 128],
                           conv_out[gt], gt == 0, idx == nmm)
                    if pw2br_s is not None:
                        mm(ps, pw2br_s[:, et * 128:(et + 1) * 128], ones_rb,
                           False, True)
                    xn = res_pool.tile([128, N], dt.float32, tag="res")
                    nc.vector.tensor_add(xn, ps, x2[et])
                    x3.append(xn)

                # ---- FF2 ----
                x4 = feed_forward2(x3, f2w1_s, f2b1c_s, f2w2_s, f2b2r_s)

                # ---- post norm + transpose out ----
                outT = layer_norm(x4, dt.float32, affine=(pngc_s, pnbc_s),
                  precise=True)
                for nt in range(NT):
                    ona = xio_pool.tile([128, D], dt.float32, tag="ona")
                    for ct in range(CT):
                        pst = ps_pool.tile([128, 128], dt.float32, tag="ps")
                        nc.tensor.transpose(
                            pst, outT[ct][:, nt * 128:(nt + 1) * 128], ident_f)
                        nc.vector.tensor_copy(
                            out=ona[:, ct * 128:(ct + 1) * 128], in_=pst)
                    nc.sync.dma_start(
                        out=out.ap()[b, nt * 128:(nt + 1) * 128, :], in_=ona)

    nc.finalize()
    return nc


class _Runner:
    """Compile once; run the 8-core shard_map NEFF repeatedly."""

    def __init__(self, nc):
        import jax
        import concourse.mybir as mybir
        from concourse import bass2jax
        from jax.sharding import Mesh, PartitionSpec, NamedSharding
        from jax.experimental.shard_map import shard_map

        bass2jax.install_neuronx_cc_hook()
        self.jax = jax
        part_name = (nc.partition_id_tensor.name
                     if nc.partition_id_tensor else None)
        in_names, out_names, out_avals, zero_outs = [], [], [], []
        for alloc in nc.m.functions[0].allocations:
            if not isinstance(alloc, mybir.MemoryLocationSet):
                continue
            name = alloc.memorylocations[0].name
            if alloc.kind == "ExternalInput":
                if name != part_name:
                    in_names.append(name)
            elif alloc.kind == "ExternalOutput":
                shape = tuple(alloc.tensor_shape)
                dtype = mybir.dt.np(alloc.dtype)
                out_names.append(name)
                out_avals.append(jax.core.ShapedArray(shape, dtype))
                zero_outs.append(np.zeros(shape, dtype))
        self.in_names, self.out_names = in_names, out_names
        self.out_avals, self.zero_outs = out_avals, zero_outs
        n_params, n_outs = len(in_names), len(out_names)
        all_names = tuple(in_names + out_names
                          + ([part_name] if part_name else []))

        def _body(*args):
            operands = list(args)
            if part_name is not None:
                operands.append(bass2jax.partition_id_tensor())
            outs = bass2jax._bass_exec_p.bind(
                *operands,
                out_avals=tuple(out_avals),
                in_names=all_names,
                out_names=tuple(out_names),
                lowering_input_output_aliases=(),
                sim_require_finite=True,
                sim_require_nnan=True,
                nc=nc,
            )
            return tuple(outs)

        devices = jax.devices()[:NCORES]
        assert len(devices) == NCORES
        self.mesh = Mesh(np.asarray(devices), ("core",))
        self.sharding = NamedSharding(self.mesh, PartitionSpec("core"))
        donate = tuple(range(n_params, n_params + n_outs))
        self.fn = jax.jit(
            shard_map(_body, mesh=self.mesh,
                      in_specs=(PartitionSpec("core"),) * (n_params + n_outs),
                      out_specs=(PartitionSpec("core"),) * n_outs,
                      check_rep=False),
            donate_argnums=donate, keep_unused=True)

    def stage_inputs(self, in_maps):
        jax = self.jax
        concat = [
            np.concatenate([np.asarray(in_maps[c][n])
                            for c in range(NCORES)], axis=0)
            for n in self.in_names
        ]
        return [jax.device_put(a, self.sharding) for a in concat]

    def fresh_zeros(self):
        jax = self.jax
        return [
            jax.device_put(
                np.zeros((NCORES * z.shape[0], *z.shape[1:]), z.dtype),
                self.sharding)
            for z in self.zero_outs
        ]

    def run(self, staged):
        outs = self.fn(*staged, *self.fresh_zeros())
        return [np.asarray(o) for o in outs]


def _get_runner(flags):
    key = tuple(sorted(flags.items()))
    if key not in _BUILD_CACHE:
        nc = _build(flags, 0, 0)
        _BUILD_CACHE[key] = _Runner(nc)
    return _BUILD_CACHE[key]


def kernel(**inputs):
    prep, flags = _host_prep(inputs)
    x_full = np.asarray(inputs["x"], np.float32)

    runner = _get_runner(flags)
    in_maps = []
    for c in range(NCORES):
        m = dict(prep)
        m["xin"] = np.ascontiguousarray(x_full[c * BL:(c + 1) * BL])
        in_maps.append(m)
    staged = runner.stage_inputs(in_maps)
    outs = runner.run(staged)

    bench_iters = int(os.environ.get("TRN_KERNEL_BENCH", "0"))
    if bench_iters:
        import time
        jax = runner.jax
        # warm
        r = runner.fn(*staged, *runner.fresh_zeros())
        jax.block_until_ready(r)
        zero_sets = [runner.fresh_zeros() for _ in range(bench_iters)]
        t0 = time.perf_counter()
        rs = [runner.fn(*staged, *zs) for zs in zero_sets]
        jax.block_until_ready(rs)
        dt_ns = (time.perf_counter() - t0) / bench_iters * 1e9
        print(f"HW exec time: {dt_ns:.0f} ns")
        # serial (non-pipelined) estimate
        t0 = time.perf_counter()
        for zs in [runner.fresh_zeros() for _ in range(4)]:
            jax.block_until_ready(runner.fn(*staged, *zs))
        ser_ns = (time.perf_counter() - t0) / 4 * 1e9
        print(f"serial exec time: {ser_ns:.0f} ns")

    oi = runner.out_names.index("out")
    full = outs[oi].reshape(NCORES, BL, N, D).reshape(B, N, D)
    return full.astype(np.float32)


if __name__ == "__main__":
    # smoke test with random inputs shaped like the reference
    rng = np.random.default_rng(0)
    print("building only...")
    p, flags = _host_prep({
        k: rng.standard_normal(s).astype(np.float32) * 0.02
        for k, s in {
            "x": (B, N, D), "ff1_ln_g": (D,), "ff1_ln_b": (D,),
            "ff1_w1": (D, FF), "ff1_b1": (FF,), "ff1_w2": (FF, D),
            "ff1_b2": (D,), "attn_ln_g": (D,), "attn_ln_b": (D,),
            "qkv_w": (D, 3 * H * DH), "out_w": (H * DH, D), "out_b": (D,),
            "rel_emb": (2 * MAXP + 1, DH), "conv_ln_g": (D,),
            "conv_ln_b": (D,), "pw1_w": (D, 2 * IC), "pw1_b": (2 * IC,),
            "dw_w": (IC, 1, KW), "dw_b": (IC,), "bn_g": (IC,), "bn_b": (IC,),
            "bn_mean": (IC,), "bn_var": (IC,), "pw2_w": (IC, D),
            "pw2_b": (D,), "ff2_ln_g": (D,), "ff2_ln_b": (D,),
            "ff2_w1": (D, FF), "ff2_b1": (FF,), "ff2_w2": (FF, D),
            "ff2_b2": (D,), "pn_g": (D,), "pn_b": (D,),
        }.items()})
    nc = _build(flags, 0, 0)
    print("build OK:", len(nc.m.functions[0].instructions)
          if hasattr(nc.m.functions[0], "instructions") else "done")
